# revision 3
# baseline (speedup 1.0000x reference)
"""Trainium2 Bass kernel for nn_DotProductAttention_50010599194781.

Computes, per (batch*head) bh:
    S = Q @ K^T / sqrt(d)                       [L, L]
    P = softmax(mask(S))                         (mask: key index >= valid_lens[bh] -> -1e6)
    mult = (q_mult[b] @ kv_mult[b]^T) / (||.||_F + 1e-5)   (per batch b, repeated over heads)
    out = (P + mult) @ V
Sharding: BH=32 heads split 4-per-core across 8 NeuronCores (SPMD, one program).

Device algorithm (per core):
  - the additive mult term is low-rank: mult @ V_h = (q_mult/fro) @ (kv_mult^T V_h).
    Both factors are tiny, so the whole term ([L, d] per head, "om") is
    precomputed on the host in f32/f64 (~2 GFLOP of BLAS) and shipped as a
    2 MB bf16 tensor; the device adds it in the fused epilogue op.
  - softmax part computed in transposed layout S^T[k, q] so exp(S^T) tiles are
    directly usable as matmul stationary weights for P @ V (no on-chip
    transposes). The ACT engine runs nothing but Exp -> a single table load.
  - masking is pure host-side data: V is pre-masked per head and an appended
    ones-column (also masked) yields the softmax denominator for free inside
    the P@V matmul. valid_len == 0 heads get scale=0 (exp->1 uniform) +
    unmasked V, matching jax softmax.
"""

import numpy as np
import ml_dtypes

import concourse.bass as bass
import concourse.tile as tile
from concourse import bacc, mybir, bass_utils
from concourse.tile_rust import add_dep_helper

B, H, L, D = 2, 16, 2048, 128
NCORES = 8
HPC = 4            # heads per core
NT = L // 128      # 16 k-tiles of 128
NJB = 4            # q-blocks of 512
PADC = 132         # PV rhs cols: 128 V + 1 ones + 3 pad (bf16)

f32 = mybir.dt.float32
f32r = mybir.dt.float32r
bf16 = mybir.dt.bfloat16
f16 = mybir.dt.float16

MULT = mybir.AluOpType.mult
ADD = mybir.AluOpType.add
EXP = mybir.ActivationFunctionType.Exp

S_DT = {'f32r': f32r, 'bf16': bf16, 'f16': f16}

DEFAULT_CFG = dict(
    tg=2,            # k-tiles per exp-activation group
    s_dt='f16',      # Q/K storage + S matmul dtype
    sps_bufs=3,      # score-PSUM pool bufs
    pvs_bufs=2,      # PV-PSUM pool bufs
    ep_bufs=5,       # exp SBUF pool bufs
    kq_bufs=2,       # per-head K/Q SBUF double buffering
    vx_bufs=2,       # per-head V SBUF double buffering
    ob_bufs=3,       # output staging bufs
    sw_pipe=True,    # emit S/exp of group t+1 before PV of group t
    split_act=False, # one act instruction per k-tile instead of per group
)


def _build_body(nc, tc, kt_d, qt_d, vx_d, om_d, sc_d, out_d, tvs, cfg):
    TG = cfg['tg']
    s_dt = S_DT[cfg['s_dt']]
    with tc.tile_pool(name="pers", bufs=1) as pers:
        scales_sb = pers.tile([128, HPC], f32)
        nc.sync.dma_start(out=scales_sb, in_=sc_d)
        omall_sb = pers.tile([128, NT, HPC * 128], bf16, name="omall_sb")

        with tc.tile_pool(name="kq", bufs=cfg['kq_bufs']) as kq, \
             tc.tile_pool(name="vxp", bufs=cfg['vx_bufs']) as vxp, \
             tc.tile_pool(name="ep", bufs=cfg['ep_bufs']) as ep, \
             tc.tile_pool(name="ob", bufs=cfg['ob_bufs']) as ob, \
             tc.tile_pool(name="sm", bufs=4) as sm, \
             tc.tile_pool(name="sps", bufs=cfg['sps_bufs'], space="PSUM") as sps, \
             tc.tile_pool(name="pvs", bufs=cfg['pvs_bufs'], space="PSUM") as pvs:
            for h in range(HPC):
                tv = tvs[h]
                kt_sb = kq.tile([128, L], s_dt, tag="kt")
                nc.sync.dma_start(out=kt_sb[:, 0:tv * 128], in_=kt_d[h][:, 0:tv * 128])
                qt_sb = kq.tile([128, L], s_dt, tag="qt")
                nc.sync.dma_start(out=qt_sb, in_=qt_d[h])
                vx_sb = vxp.tile([128, NT, PADC], bf16, tag="vx")
                nc.sync.dma_start(
                    out=vx_sb[:, 0:tv, :],
                    in_=vx_d[h].rearrange("p (t c) -> p t c", t=NT)[:, 0:tv, :])
                if h == 0:
                    # the mult-term tensor rides behind head 0's tiles: it is
                    # first read by the (h0, jb0) epilogue, ~25us in.
                    nc.sync.dma_start(
                        out=omall_sb,
                        in_=om_d.rearrange("p (t c) -> p t c", t=NT))

                for jb in range(NJB):
                    # two PSUM banks, each packing two q-subtile accumulators
                    # (cols [0:PADC] and [256:256+PADC]); col 128 of each region is
                    # the softmax denominator (ones column of vx).
                    pv0 = pvs.tile([128, 512], f32, tag="pv", name="pv0")
                    pv1 = pvs.tile([128, 512], f32, tag="pv", name="pv1")
                    pvb = [pv0, pv1]
                    pv_first = [None, None]

                    def emit_pv(tg, tls, exps):
                        for tl in tls:
                            t = TG * tg + tl
                            for qs in range(4):
                                bank = pvb[qs // 2]
                                off = (qs % 2) * 256
                                # has_written packing: the first matmul emitted into a
                                # bank uses start=True (clears the whole bank's bits);
                                # the other region's t=0 matmul relies on its bits being
                                # clear -> plain write, then accumulates for t>=1.
                                st = (t == 0 and qs % 2 == 0)
                                mm = nc.tensor.matmul(
                                    bank[:, off:off + PADC],
                                    lhsT=exps[:, tl * 512 + qs * 128:
                                              tl * 512 + (qs + 1) * 128],
                                    rhs=vx_sb[:, t, :],
                                    start=st,
                                    stop=(t == tv - 1 and qs % 2 == 1),
                                    skip_group_check=True)
                                if t == 0:
                                    if qs % 2 == 0:
                                        pv_first[qs // 2] = mm
                                    else:
                                        add_dep_helper(
                                            mm.ins, pv_first[qs // 2].ins,
                                            sync=False,
                                            reason="psum has_written bank packing")

                    # software-pipelined emission: S/exp of group tg are emitted
                    # BEFORE the PV matmuls of group tg-1, so the in-order PE
                    # stream always has score matmuls to chew on while ACT is
                    # still producing the previous group's exp.
                    pend = None
                    for tg in range((tv + TG - 1) // TG):
                        tls = [i for i in range(TG) if TG * tg + i < tv]
                        width = 512 * len(tls)
                        sp_t = sps.tile([128, 512 * TG], f32, tag="sp", name="sp_t")
                        for tl in tls:
                            t = TG * tg + tl
                            nc.tensor.matmul(
                                sp_t[:, tl * 512:(tl + 1) * 512],
                                lhsT=kt_sb[:, t * 128:(t + 1) * 128],
                                rhs=qt_sb[:, jb * 512:(jb + 1) * 512],
                                start=True, stop=True)
                        exps = ep.tile([128, 512 * TG], bf16, tag="exps", name="exps")
                        if cfg['split_act']:
                            for tl in tls:
                                nc.scalar.activation(
                                    exps[:, tl * 512:(tl + 1) * 512],
                                    sp_t[:, tl * 512:(tl + 1) * 512], EXP,
                                    scale=scales_sb[:, h:h + 1])
                        else:
                            nc.scalar.activation(exps[:, 0:width], sp_t[:, 0:width],
                                                 EXP, scale=scales_sb[:, h:h + 1])
                        if not cfg['sw_pipe']:
                            emit_pv(tg, tls, exps)
                        else:
                            if pend is not None:
                                emit_pv(*pend)
                            pend = (tg, tls, exps)
                    if pend is not None:
                        emit_pv(*pend)

                    # epilogue: one merged reciprocal per bank ([128,2] over the
                    # two denominator columns), then fused (pv*rec)+om per qs.
                    recs = []
                    for bank in pvb:
                        rec2 = sm.tile([128, 2], f32, tag="rec", name="rec2")
                        nc.vector.reciprocal(
                            rec2, bank.rearrange("p (r c) -> p r c", r=2)[:, :, 128])
                        recs.append(rec2)

                    osb = ob.tile([128, 4, 128], f32, tag="osb", name="osb")
                    for qs in range(4):
                        bank = pvb[qs // 2]
                        off = (qs % 2) * 256
                        nc.vector.scalar_tensor_tensor(
                            out=osb[:, qs, :],
                            in0=bank[:, off:off + 128],
                            scalar=recs[qs // 2][:, (qs % 2):(qs % 2) + 1],
                            in1=omall_sb[:, jb * 4 + qs, h * 128:(h + 1) * 128],
                            op0=MULT, op1=ADD)
                    nc.sync.dma_start(
                        out=out_d[h, jb * 512:(jb + 1) * 512, :]
                            .rearrange("(s p) d -> p s d", p=128),
                        in_=osb)


def build_program(repeat: int = 1, tvs=(NT,) * HPC, cfg=None):
    cfg = dict(DEFAULT_CFG, **(cfg or {}))
    nc = bacc.Bacc("TRN2", target_bir_lowering=False, debug=False,
                   enable_asserts=False, num_devices=NCORES)
    s_dt = S_DT[cfg['s_dt']]
    kt_d = nc.dram_tensor("kt", (HPC, 128, L), s_dt, kind="ExternalInput").ap()
    qt_d = nc.dram_tensor("qt", (HPC, 128, L), s_dt, kind="ExternalInput").ap()
    vx_d = nc.dram_tensor("vx", (HPC, 128, NT * PADC), bf16, kind="ExternalInput").ap()
    om_d = nc.dram_tensor("om", (128, NT * HPC * 128), bf16, kind="ExternalInput").ap()
    sc_d = nc.dram_tensor("sc", (128, HPC), f32, kind="ExternalInput").ap()
    out_d = nc.dram_tensor("out", (HPC, L, D), f32, kind="ExternalOutput").ap()

    with tile.TileContext(nc) as tc:
        for _ in range(repeat):
            _build_body(nc, tc, kt_d, qt_d, vx_d, om_d, sc_d, out_d, tvs, cfg)
    nc.compile()
    return nc


def head_order_and_tvs(valid_lens):
    # Assign heads to (core, slot) with cross-core balancing WITHIN each batch
    # (the mult term is per-batch, so any in-batch shuffle is free):
    # batch b's heads sorted desc by k-tile count; rank r -> core 4b + r%4,
    # slot r//4. Slot trip count = max over both batches of rank 4s.
    vl = np.asarray(valid_lens).astype(np.int64)
    tv_all = np.where(vl == 0, NT, -(-vl // 128)).astype(int)
    n_batches = NCORES * HPC // H
    cores_per_batch = NCORES // n_batches
    assign = np.zeros((NCORES, HPC), int)
    for b in range(n_batches):
        idxs = np.arange(b * H, (b + 1) * H)
        ranked = idxs[np.argsort(-tv_all[idxs], kind="stable")]
        for r, bh in enumerate(ranked):
            assign[b * cores_per_batch + r % cores_per_batch][r // cores_per_batch] = bh
    tvs = tuple(int(max(tv_all[assign[c][s]] for c in range(NCORES)))
                for s in range(HPC))
    return assign, tvs


def host_prepare(queries, keys, values, q_mult, kv_mult, valid_lens, num_heads,
                 order=None, s_dt='f16'):
    queries = np.asarray(queries, dtype=np.float32)
    keys = np.asarray(keys, dtype=np.float32)
    values = np.asarray(values, dtype=np.float32)
    q_mult = np.asarray(q_mult, dtype=np.float32)
    kv_mult = np.asarray(kv_mult, dtype=np.float32)
    valid_lens = np.asarray(valid_lens).astype(np.int64)
    np_sdt = {'f32r': np.float32, 'bf16': ml_dtypes.bfloat16, 'f16': np.float16}[s_dt]

    # Mult term on the host. Frobenius norm via the Gram trick in f64:
    # ||A B^T||_F^2 = sum((A^T A) * (B^T B)). Then
    # om[bh] = (q_mult[b]/ (fro+eps)) @ (kv_mult[b]^T @ V[bh])  -- [L, D] each.
    omf = np.empty((B * H, L, D), np.float32)
    for b in range(B):
        gq = q_mult[b].astype(np.float64).T @ q_mult[b].astype(np.float64)
        gk = kv_mult[b].astype(np.float64).T @ kv_mult[b].astype(np.float64)
        fro = np.sqrt(np.sum(gq * gk))
        qm = (q_mult[b] / np.float32(fro + 1e-5))
        tsx = np.matmul(kv_mult[b].T[None], values[b * H:(b + 1) * H])
        omf[b * H:(b + 1) * H] = np.matmul(qm[None], tsx)

    if order is None:
        order = np.arange(NCORES * HPC).reshape(NCORES, HPC)
    in_maps = []
    for c in range(NCORES):
        idx = [int(order[c][i]) for i in range(HPC)]
        kt = np.ascontiguousarray(keys[idx].transpose(0, 2, 1)).astype(np_sdt)
        qt = np.ascontiguousarray(queries[idx].transpose(0, 2, 1)).astype(np_sdt)

        vx = np.zeros((HPC, L, PADC), np.float32)
        sc = np.zeros((128, HPC), np.float32)
        for i in range(HPC):
            v = int(valid_lens[idx[i]])
            if v == 0:
                vx[i, :, 0:D] = values[idx[i]]
                vx[i, :, D] = 1.0
                sc[:, i] = 0.0
            else:
                m = (np.arange(L) < v).astype(np.float32)
                vx[i, :, 0:D] = values[idx[i]] * m[:, None]
                vx[i, :, D] = m
                sc[:, i] = 1.0 / np.sqrt(float(D))
        vxr = np.ascontiguousarray(
            vx.reshape(HPC, NT, 128, PADC).transpose(0, 2, 1, 3)
              .reshape(HPC, 128, NT * PADC)).astype(ml_dtypes.bfloat16)

        # om device layout: om_d[p, qg, (h,d)] = omf[head h][qg*128+p, d]
        om = np.ascontiguousarray(
            omf[idx].reshape(HPC, NT, 128, D).transpose(2, 1, 0, 3)
                    .reshape(128, NT * HPC * D)).astype(ml_dtypes.bfloat16)

        in_maps.append(dict(kt=kt, qt=qt, vx=vxr, om=om, sc=sc))
    return in_maps


_PROGRAM_CACHE = {}


def _get_program(repeat: int = 1, tvs=(NT,) * HPC, cfg=None):
    key = (repeat, tuple(tvs), tuple(sorted((cfg or {}).items())))
    if key not in _PROGRAM_CACHE:
        _PROGRAM_CACHE[key] = build_program(repeat, tvs, cfg)
    return _PROGRAM_CACHE[key]


def kernel(queries, keys, values, q_mult, kv_mult, valid_lens, num_heads, **_unused):
    num_heads = int(np.asarray(num_heads))
    order, tvs = head_order_and_tvs(valid_lens)
    in_maps = host_prepare(queries, keys, values, q_mult, kv_mult, valid_lens,
                           num_heads, order, s_dt=DEFAULT_CFG['s_dt'])
    nc = _get_program(1, tvs)
    res = None
    for attempt in range(3):
        try:
            res = bass_utils.run_bass_kernel_spmd(
                nc, in_maps, core_ids=list(range(NCORES)))
            break
        except Exception:
            if attempt == 2:
                raise
            import time as _time
            _time.sleep(5)
    out = np.empty((NCORES * HPC, L, D), np.float32)
    for c in range(NCORES):
        o = np.asarray(res.results[c]["out"], np.float32)
        for i in range(HPC):
            out[int(order[c][i])] = o[i]
    return out


# revision 4
# speedup vs baseline: 1.2360x; 1.2360x over previous
"""Trainium2 Bass kernel for nn_DotProductAttention_50010599194781.

Computes, per (batch*head) bh:
    S = Q @ K^T / sqrt(d)                       [L, L]
    P = softmax(mask(S))                         (mask: key index >= valid_lens[bh] -> -1e6)
    mult = (q_mult[b] @ kv_mult[b]^T) / (||.||_F + 1e-5)   (per batch b, repeated over heads)
    out = (P + mult) @ V
Sharding: BH=32 heads split 4-per-core across 8 NeuronCores.

Per-core PROGRAMS (not one SPMD stream): the per-head k-tile trip counts are
compile-time constants, so a shared program must pad every core to the max
trip count per head-slot (42 tiles/core for this input). Compiling one
program per distinct per-core trip-count tuple removes that padding; heads
are globally load-balanced so every core does ~34 k-tiles. Cores with equal
trip tuples share a program and run in one run_bass_kernel_spmd group.

Device algorithm (per core):
  - the additive mult term is low-rank: mult @ V_h = (q_mult/fro) @ (kv_mult^T V_h).
    Both factors are tiny, so the whole term ([L, d] per head, "om") is
    precomputed on the host in f32/f64 (~2 GFLOP of BLAS) and shipped as a
    2 MB bf16 tensor; the device adds it in the fused epilogue op. With the
    mult term off-device there is no per-batch work left, so any core can
    take any head mix (this is what makes global balancing legal).
  - softmax part computed in transposed layout S^T[k, q] so exp(S^T) tiles are
    directly usable as matmul stationary weights for P @ V (no on-chip
    transposes). The ACT engine runs nothing but Exp -> a single table load.
  - masking is pure host-side data: V is pre-masked per head and an appended
    ones-column (also masked) yields the softmax denominator for free inside
    the P@V matmul. valid_len == 0 heads get scale=0 (exp->1 uniform) +
    unmasked V, matching jax softmax.
"""

import numpy as np
import ml_dtypes

import concourse.bass as bass
import concourse.tile as tile
from concourse import bacc, mybir, bass_utils
from concourse.tile_rust import add_dep_helper

B, H, L, D = 2, 16, 2048, 128
NCORES = 8
HPC = 4            # heads per core
NT = L // 128      # 16 k-tiles of 128
NJB = 4            # q-blocks of 512
PADC = 132         # PV rhs cols: 128 V + 1 ones + 3 pad (bf16)

f32 = mybir.dt.float32
f32r = mybir.dt.float32r
bf16 = mybir.dt.bfloat16
f16 = mybir.dt.float16

MULT = mybir.AluOpType.mult
ADD = mybir.AluOpType.add
EXP = mybir.ActivationFunctionType.Exp

S_DT = {'f32r': f32r, 'bf16': bf16, 'f16': f16}

DEFAULT_CFG = dict(
    tg=2,            # k-tiles per exp-activation group
    s_dt='f16',      # Q/K storage + S matmul dtype
    sps_bufs=3,      # score-PSUM pool bufs
    pvs_bufs=2,      # PV-PSUM pool bufs
    ep_bufs=5,       # exp SBUF pool bufs
    kq_bufs=2,       # per-head K/Q SBUF double buffering
    vx_bufs=2,       # per-head V SBUF double buffering
    ob_bufs=3,       # output staging bufs
    sw_pipe=True,    # emit S/exp of group t+1 before PV of group t
    split_act=False, # one act instruction per k-tile instead of per group
)


def _build_body(nc, tc, kt_d, qt_d, vx_d, om_d, sc_d, out_d, tvs, cfg):
    TG = cfg['tg']
    s_dt = S_DT[cfg['s_dt']]
    with tc.tile_pool(name="pers", bufs=1) as pers:
        scales_sb = pers.tile([128, HPC], f32)
        nc.sync.dma_start(out=scales_sb, in_=sc_d)
        omall_sb = pers.tile([128, NT, HPC * 128], bf16, name="omall_sb")

        with tc.tile_pool(name="kq", bufs=cfg['kq_bufs']) as kq, \
             tc.tile_pool(name="vxp", bufs=cfg['vx_bufs']) as vxp, \
             tc.tile_pool(name="ep", bufs=cfg['ep_bufs']) as ep, \
             tc.tile_pool(name="ob", bufs=cfg['ob_bufs']) as ob, \
             tc.tile_pool(name="sm", bufs=4) as sm, \
             tc.tile_pool(name="sps", bufs=cfg['sps_bufs'], space="PSUM") as sps, \
             tc.tile_pool(name="pvs", bufs=cfg['pvs_bufs'], space="PSUM") as pvs:
            for h in range(HPC):
                tv = tvs[h]
                kt_sb = kq.tile([128, L], s_dt, tag="kt")
                qt_sb = kq.tile([128, L], s_dt, tag="qt")
                vx_sb = vxp.tile([128, NT, PADC], bf16, tag="vx")
                if h == 0:
                    # split the first head's loads so the first S-group (k-tiles
                    # 0..1 x q-block 0) can start after ~0.3 MB instead of 2.4 MB.
                    t0 = min(TG, tv)
                    nc.sync.dma_start(out=kt_sb[:, 0:t0 * 128],
                                      in_=kt_d[h][:, 0:t0 * 128])
                    nc.sync.dma_start(out=qt_sb[:, 0:512], in_=qt_d[h][:, 0:512])
                    if tv > t0:
                        nc.sync.dma_start(out=kt_sb[:, t0 * 128:tv * 128],
                                          in_=kt_d[h][:, t0 * 128:tv * 128])
                    nc.sync.dma_start(out=qt_sb[:, 512:L], in_=qt_d[h][:, 512:L])
                    vx_r = vx_d[h].rearrange("p (t c) -> p t c", t=NT)
                    nc.sync.dma_start(out=vx_sb[:, 0:t0, :], in_=vx_r[:, 0:t0, :])
                    if tv > t0:
                        nc.sync.dma_start(out=vx_sb[:, t0:tv, :],
                                          in_=vx_r[:, t0:tv, :])
                    # the mult-term tensor rides behind head 0's tiles: it is
                    # first read by the (h0, jb0) epilogue, ~20us in.
                    nc.sync.dma_start(
                        out=omall_sb,
                        in_=om_d.rearrange("p (t c) -> p t c", t=NT))
                else:
                    nc.sync.dma_start(out=kt_sb[:, 0:tv * 128],
                                      in_=kt_d[h][:, 0:tv * 128])
                    nc.sync.dma_start(out=qt_sb, in_=qt_d[h])
                    nc.sync.dma_start(
                        out=vx_sb[:, 0:tv, :],
                        in_=vx_d[h].rearrange("p (t c) -> p t c", t=NT)[:, 0:tv, :])

                for jb in range(NJB):
                    # two PSUM banks, each packing two q-subtile accumulators
                    # (cols [0:PADC] and [256:256+PADC]); col 128 of each region is
                    # the softmax denominator (ones column of vx).
                    pv0 = pvs.tile([128, 512], f32, tag="pv", name="pv0")
                    pv1 = pvs.tile([128, 512], f32, tag="pv", name="pv1")
                    pvb = [pv0, pv1]
                    pv_first = [None, None]

                    def emit_pv(tg, tls, exps):
                        for tl in tls:
                            t = TG * tg + tl
                            for qs in range(4):
                                bank = pvb[qs // 2]
                                off = (qs % 2) * 256
                                # has_written packing: the first matmul emitted into a
                                # bank uses start=True (clears the whole bank's bits);
                                # the other region's t=0 matmul relies on its bits being
                                # clear -> plain write, then accumulates for t>=1.
                                st = (t == 0 and qs % 2 == 0)
                                mm = nc.tensor.matmul(
                                    bank[:, off:off + PADC],
                                    lhsT=exps[:, tl * 512 + qs * 128:
                                              tl * 512 + (qs + 1) * 128],
                                    rhs=vx_sb[:, t, :],
                                    start=st,
                                    stop=(t == tv - 1 and qs % 2 == 1),
                                    skip_group_check=True)
                                if t == 0:
                                    if qs % 2 == 0:
                                        pv_first[qs // 2] = mm
                                    else:
                                        add_dep_helper(
                                            mm.ins, pv_first[qs // 2].ins,
                                            sync=False,
                                            reason="psum has_written bank packing")

                    # software-pipelined emission: S/exp of group tg are emitted
                    # BEFORE the PV matmuls of group tg-1, so the in-order PE
                    # stream always has score matmuls to chew on while ACT is
                    # still producing the previous group's exp.
                    pend = None
                    for tg in range((tv + TG - 1) // TG):
                        tls = [i for i in range(TG) if TG * tg + i < tv]
                        width = 512 * len(tls)
                        sp_t = sps.tile([128, 512 * TG], f32, tag="sp", name="sp_t")
                        for tl in tls:
                            t = TG * tg + tl
                            nc.tensor.matmul(
                                sp_t[:, tl * 512:(tl + 1) * 512],
                                lhsT=kt_sb[:, t * 128:(t + 1) * 128],
                                rhs=qt_sb[:, jb * 512:(jb + 1) * 512],
                                start=True, stop=True)
                        exps = ep.tile([128, 512 * TG], bf16, tag="exps", name="exps")
                        if cfg['split_act']:
                            for tl in tls:
                                nc.scalar.activation(
                                    exps[:, tl * 512:(tl + 1) * 512],
                                    sp_t[:, tl * 512:(tl + 1) * 512], EXP,
                                    scale=scales_sb[:, h:h + 1])
                        else:
                            nc.scalar.activation(exps[:, 0:width], sp_t[:, 0:width],
                                                 EXP, scale=scales_sb[:, h:h + 1])
                        if not cfg['sw_pipe']:
                            emit_pv(tg, tls, exps)
                        else:
                            if pend is not None:
                                emit_pv(*pend)
                            pend = (tg, tls, exps)
                    if pend is not None:
                        emit_pv(*pend)

                    # epilogue: one merged reciprocal per bank ([128,2] over the
                    # two denominator columns), then fused (pv*rec)+om per qs.
                    recs = []
                    for bank in pvb:
                        rec2 = sm.tile([128, 2], f32, tag="rec", name="rec2")
                        nc.vector.reciprocal(
                            rec2, bank.rearrange("p (r c) -> p r c", r=2)[:, :, 128])
                        recs.append(rec2)

                    osb = ob.tile([128, 4, 128], f32, tag="osb", name="osb")
                    for qs in range(4):
                        bank = pvb[qs // 2]
                        off = (qs % 2) * 256
                        nc.vector.scalar_tensor_tensor(
                            out=osb[:, qs, :],
                            in0=bank[:, off:off + 128],
                            scalar=recs[qs // 2][:, (qs % 2):(qs % 2) + 1],
                            in1=omall_sb[:, jb * 4 + qs, h * 128:(h + 1) * 128],
                            op0=MULT, op1=ADD)
                    nc.sync.dma_start(
                        out=out_d[h, jb * 512:(jb + 1) * 512, :]
                            .rearrange("(s p) d -> p s d", p=128),
                        in_=osb)


def build_program(repeat: int = 1, tvs=(NT,) * HPC, cfg=None):
    cfg = dict(DEFAULT_CFG, **(cfg or {}))
    nc = bacc.Bacc("TRN2", target_bir_lowering=False, debug=False,
                   enable_asserts=False, num_devices=NCORES)
    s_dt = S_DT[cfg['s_dt']]
    kt_d = nc.dram_tensor("kt", (HPC, 128, L), s_dt, kind="ExternalInput").ap()
    qt_d = nc.dram_tensor("qt", (HPC, 128, L), s_dt, kind="ExternalInput").ap()
    vx_d = nc.dram_tensor("vx", (HPC, 128, NT * PADC), bf16, kind="ExternalInput").ap()
    om_d = nc.dram_tensor("om", (128, NT * HPC * 128), bf16, kind="ExternalInput").ap()
    sc_d = nc.dram_tensor("sc", (128, HPC), f32, kind="ExternalInput").ap()
    out_d = nc.dram_tensor("out", (HPC, L, D), f32, kind="ExternalOutput").ap()

    with tile.TileContext(nc) as tc:
        for _ in range(repeat):
            _build_body(nc, tc, kt_d, qt_d, vx_d, om_d, sc_d, out_d, tvs, cfg)
    nc.compile()
    return nc


def head_order_and_tvs(valid_lens):
    """Globally balance heads across cores: greedy longest-processing-time
    assignment of k-tile counts + pairwise-swap refinement, minimizing the
    max per-core tile sum. Returns (assign [8,4] of head ids, list of 8
    per-core trip tuples sorted desc)."""
    vl = np.asarray(valid_lens).astype(np.int64)
    tv_all = np.where(vl == 0, NT, -(-vl // 128)).astype(int)
    n = len(vl)
    order = np.argsort(-tv_all, kind="stable")
    cores = [[] for _ in range(NCORES)]
    sums = np.zeros(NCORES, int)
    for bh in order:
        cand = [c for c in range(NCORES) if len(cores[c]) < HPC]
        c = min(cand, key=lambda c: (sums[c], len(cores[c])))
        cores[c].append(int(bh))
        sums[c] += tv_all[bh]
    # pairwise swap refinement: reduce (max, sum of squares)
    for _ in range(100):
        improved = False
        cmax = int(np.argmax(sums))
        for co in range(NCORES):
            if co == cmax:
                continue
            for i in range(HPC):
                for j in range(HPC):
                    a, b_ = cores[cmax][i], cores[co][j]
                    d = tv_all[a] - tv_all[b_]
                    if d <= 0:
                        continue
                    new_max = sums[cmax] - d
                    new_co = sums[co] + d
                    if max(new_max, new_co) < sums[cmax]:
                        cores[cmax][i], cores[co][j] = b_, a
                        sums[cmax] -= d
                        sums[co] += d
                        improved = True
                        break
                if improved:
                    break
            if improved:
                break
        if not improved:
            break
    assign = np.zeros((NCORES, HPC), int)
    tvs_per_core = []
    for c in range(NCORES):
        hs = sorted(cores[c], key=lambda h: -tv_all[h])
        assign[c] = hs
        tvs_per_core.append(tuple(int(tv_all[h]) for h in hs))
    return assign, tvs_per_core


def host_prepare(queries, keys, values, q_mult, kv_mult, valid_lens, num_heads,
                 order=None, s_dt='f16'):
    queries = np.asarray(queries, dtype=np.float32)
    keys = np.asarray(keys, dtype=np.float32)
    values = np.asarray(values, dtype=np.float32)
    q_mult = np.asarray(q_mult, dtype=np.float32)
    kv_mult = np.asarray(kv_mult, dtype=np.float32)
    valid_lens = np.asarray(valid_lens).astype(np.int64)
    np_sdt = {'f32r': np.float32, 'bf16': ml_dtypes.bfloat16, 'f16': np.float16}[s_dt]

    # Mult term on the host. Frobenius norm via the Gram trick in f64:
    # ||A B^T||_F^2 = sum((A^T A) * (B^T B)). Then
    # om[bh] = (q_mult[b]/(fro+eps)) @ (kv_mult[b]^T @ V[bh])  -- [L, D] each.
    omf = np.empty((B * H, L, D), np.float32)
    for b in range(B):
        gq = q_mult[b].astype(np.float64).T @ q_mult[b].astype(np.float64)
        gk = kv_mult[b].astype(np.float64).T @ kv_mult[b].astype(np.float64)
        fro = np.sqrt(np.sum(gq * gk))
        qm = (q_mult[b] / np.float32(fro + 1e-5))
        tsx = np.matmul(kv_mult[b].T[None], values[b * H:(b + 1) * H])
        omf[b * H:(b + 1) * H] = np.matmul(qm[None], tsx)

    if order is None:
        order = np.arange(NCORES * HPC).reshape(NCORES, HPC)
    in_maps = []
    for c in range(NCORES):
        idx = [int(order[c][i]) for i in range(HPC)]
        kt = np.ascontiguousarray(keys[idx].transpose(0, 2, 1)).astype(np_sdt)
        qt = np.ascontiguousarray(queries[idx].transpose(0, 2, 1)).astype(np_sdt)

        vx = np.zeros((HPC, L, PADC), np.float32)
        sc = np.zeros((128, HPC), np.float32)
        for i in range(HPC):
            v = int(valid_lens[idx[i]])
            if v == 0:
                vx[i, :, 0:D] = values[idx[i]]
                vx[i, :, D] = 1.0
                sc[:, i] = 0.0
            else:
                m = (np.arange(L) < v).astype(np.float32)
                vx[i, :, 0:D] = values[idx[i]] * m[:, None]
                vx[i, :, D] = m
                sc[:, i] = 1.0 / np.sqrt(float(D))
        vxr = np.ascontiguousarray(
            vx.reshape(HPC, NT, 128, PADC).transpose(0, 2, 1, 3)
              .reshape(HPC, 128, NT * PADC)).astype(ml_dtypes.bfloat16)

        # om device layout: om_d[p, qg, (h,d)] = omf[head h][qg*128+p, d]
        om = np.ascontiguousarray(
            omf[idx].reshape(HPC, NT, 128, D).transpose(2, 1, 0, 3)
                    .reshape(128, NT * HPC * D)).astype(ml_dtypes.bfloat16)

        in_maps.append(dict(kt=kt, qt=qt, vx=vxr, om=om, sc=sc))
    return in_maps


_PROGRAM_CACHE = {}


def _get_program(repeat: int = 1, tvs=(NT,) * HPC, cfg=None):
    key = (repeat, tuple(tvs), tuple(sorted((cfg or {}).items())))
    if key not in _PROGRAM_CACHE:
        _PROGRAM_CACHE[key] = build_program(repeat, tvs, cfg)
    return _PROGRAM_CACHE[key]


def kernel(queries, keys, values, q_mult, kv_mult, valid_lens, num_heads, **_unused):
    num_heads = int(np.asarray(num_heads))
    order, tvs_pc = head_order_and_tvs(valid_lens)
    in_maps = host_prepare(queries, keys, values, q_mult, kv_mult, valid_lens,
                           num_heads, order, s_dt=DEFAULT_CFG['s_dt'])
    # group cores by identical trip tuples -> shared program per group
    groups = {}
    for c in range(NCORES):
        groups.setdefault(tvs_pc[c], []).append(c)
    results = [None] * NCORES
    for tvs, cores in groups.items():
        nc = _get_program(1, tvs)
        for attempt in range(3):
            try:
                res = bass_utils.run_bass_kernel_spmd(
                    nc, [in_maps[c] for c in cores], core_ids=cores)
                break
            except Exception:
                if attempt == 2:
                    raise
                import time as _time
                _time.sleep(5)
        for i, c in enumerate(cores):
            results[c] = res.results[i]
    out = np.empty((NCORES * HPC, L, D), np.float32)
    for c in range(NCORES):
        o = np.asarray(results[c]["out"], np.float32)
        for i in range(HPC):
            out[int(order[c][i])] = o[i]
    return out
